# revision 29
# baseline (speedup 1.0000x reference)
"""Trainium2 Bass kernel for the e3nn-style GNN convolution layer.

kernel(**inputs) takes FULL (unsharded) numpy inputs and returns the FULL
[N, 160] float32 output.  Internally shards edges across 8 NeuronCores by
destination-node range, runs one SPMD Bass program, and reassembles on host.

Design (v2):
  host prep   - fold all scalar normalizations into weights; x~ = node_input *
                node_attr; per-core node table ROTATED so the core's own slice
                comes first (gather indices remapped accordingly); edges
                grouped by (dst-window-PAIR, src-table-half, window) padded to
                th 128-edge tiles per (half, window); ALL per-pair edge
                operands packed into ONE bf16 tensor (e1 x8, e0 x8, ldst x8,
                gather idx bits).
  build phase - per 8 global windows: 2 batched xTbf loads, per window a
                fused bf16 matmul (n=320 for the first wpc windows: lin1 y +
                self-connection s; n=160 otherwise), y copied bf16 into an
                8-window staging tile, ONE batched DMA store to the y_table.
  edge phase  - per window PAIR: 1 pack DMA + 1 eleT DMA + 2 dma_gathers
                (2 swdge queues, 2ni idxs each); FC net (bf16 matmuls + Silu);
                message build on DVE/Pool all in 2x perf mode; selection
                matmuls producing TRANSPOSED z blocks directly in PSUM.
  node phase  - per window: one zT copy (ACT), lin2 (4 bf16 matmuls), add s,
                batched output store per pair.
"""

import math
from dataclasses import dataclass

import numpy as np
import ml_dtypes

import concourse.bacc as bacc
import concourse.mybir as mybir
import concourse.tile as tile
from concourse.bass_utils import run_bass_kernel_spmd
from concourse.masks import make_identity

BF16 = ml_dtypes.bfloat16
F32 = np.float32

MUL0 = 64
MUL1 = 32
FC_IN = 16
FC_H = 64
WN = 2 * MUL0 + 2 * MUL1  # 192 per-edge tp weights
D_IN = MUL0 + 3 * MUL1    # 160
DE = 256                  # padded y-table row elems (512 B in bf16)
D_MID = 4 * (MUL0 + MUL1) # 384 = [z0 (96) | z1_c0 | z1_c1 | z1_c2]
NUM_NEIGH = 10.0
C_S = math.sin(math.pi / 8.0)
C_X = math.cos(math.pi / 8.0)
P = 128


@dataclass(frozen=True)
class Cfg:
    n: int
    n_cores: int
    npc: int          # nodes per core
    wpc: int          # 128-node windows per core
    npad: int         # wpc * 128
    th: int           # tiles per (window, table-half)
    repeat: int = 1
    null: bool = False
    simsafe: bool = False  # memset staging pads (CoreSim exec strictness)
    dbg_sigmoid: bool = False  # Sigmoid instead of Silu (CoreSim exec debug)
    sim_1q: bool = False   # force both gathers onto swdge queue 0 (sim only)
    # timing ablations (break correctness)
    ab_no_build: bool = False
    ab_no_gather: bool = False
    ab_no_fc: bool = False
    ab_no_s4: bool = False
    ab_no_msg: bool = False
    ab_no_sel: bool = False
    ab_no_node: bool = False


def _to_cmajor(x_uc):
    s = x_uc.shape[:-1]
    return x_uc.reshape(*s, MUL1, 3).swapaxes(-1, -2).reshape(*s, 96)


def _from_cmajor(x_cu):
    s = x_cu.shape[:-1]
    return x_cu.reshape(*s, 3, MUL1).swapaxes(-1, -2).reshape(*s, 96)


# ---------------------------------------------------------------- host prep

def _prep(inputs, n_cores=8):
    node_input = np.asarray(inputs["node_input"], F32)
    node_attr = np.asarray(inputs["node_attr"], F32)
    edge_src = np.asarray(inputs["edge_src"]).astype(np.int64)
    edge_dst = np.asarray(inputs["edge_dst"]).astype(np.int64)
    edge_attr = np.asarray(inputs["edge_attr"], F32)
    ele = np.asarray(inputs["edge_length_embedded"], F32)

    n = node_input.shape[0]
    assert n % n_cores == 0
    npc = n // n_cores
    wpc = (npc + P - 1) // P
    npad = wpc * P
    ntab = n_cores * npad
    half = ntab // 2
    assert half <= 32767 and half % npad == 0

    inv0 = 1.0 / math.sqrt(MUL0)
    inv1 = 1.0 / math.sqrt(MUL1)
    invm = 1.0 / math.sqrt(MUL0 + MUL1)
    invnb = 1.0 / math.sqrt(NUM_NEIGH)

    x = node_input * node_attr
    xT = np.concatenate([x[:, :MUL0], _to_cmajor(x[:, MUL0:])], axis=1).T
    xT = np.ascontiguousarray(xT, F32)   # [160, n]

    W_sc0 = np.asarray(inputs["W_sc0"], F32) * (inv0 * C_S)
    W_sc1 = np.asarray(inputs["W_sc1"], F32) * (inv1 * C_S)
    W_l1_0 = np.asarray(inputs["W_l1_0"], F32) * inv0
    W_l1_1 = np.asarray(inputs["W_l1_1"], F32) * inv1
    fc_W1 = np.asarray(inputs["fc_W1"], F32) * (1.0 / math.sqrt(FC_IN))
    fc_W2 = np.asarray(inputs["fc_W2"], F32) * (1.0 / math.sqrt(FC_H))
    obase = invm * C_X * invnb
    W_l2_0 = np.asarray(inputs["W_l2_0"], F32) * obase
    W_l2_0 = W_l2_0.copy()
    W_l2_0[MUL0:, :] *= 1.0 / math.sqrt(3.0)
    W_l2_1 = np.asarray(inputs["W_l2_1"], F32) * obase

    def blockdiag(*ms):
        rows = sum(m.shape[0] for m in ms)
        cols = sum(m.shape[1] for m in ms)
        out = np.zeros((rows, cols), F32)
        r = c = 0
        for m in ms:
            out[r:r + m.shape[0], c:c + m.shape[1]] = m
            r += m.shape[0]
            c += m.shape[1]
        return out

    Wsc_big = blockdiag(W_sc0, W_sc1, W_sc1, W_sc1)
    Wl1_big = blockdiag(W_l1_0, W_l1_1, W_l1_1, W_l1_1)
    # build-loop rhs: [Wl1 (160) | Wsc (160)] — s half read for local windows
    Wb = np.ascontiguousarray(np.concatenate([Wl1_big, Wsc_big], axis=1), BF16)

    # ---- edge sharding
    core = edge_dst // npc
    local = edge_dst - core * npc
    win = local // P
    ldst = (local - win * P).astype(F32)
    src_remap = (edge_src // npc) * npad + (edge_src % npc)

    # per-core rotated table: core k's table row = (global_remap - k*npad) % ntab
    cnt2_all = np.zeros((n_cores, wpc * 2), np.int64)
    percore = []
    for k in range(n_cores):
        m = core == k
        rsrc = (src_remap[m] - k * npad) % ntab
        hbit = (rsrc >= half).astype(np.int64)
        g2 = win[m] * 2 + hbit
        percore.append((m, rsrc, hbit, g2))
        cnt2_all[k] = np.bincount(g2, minlength=wpc * 2)
    th = max(1, int((cnt2_all.max() + P - 1) // P))
    ni = th * P
    ni16 = ni // 16
    tt = 2 * th            # tiles per (window)
    T2 = 2 * tt            # tiles per full pair
    npairs = (wpc + 1) // 2
    flat = npairs * T2 * P  # total slots (tail pair padded to a full pair)
    ep = flat

    xT_pad = np.zeros((n_cores, D_IN, npad), F32)
    for k in range(n_cores):
        xT_pad[k, :, :npc] = xT[:, k * npc:(k + 1) * npc]
    xTbf_global = np.ascontiguousarray(
        xT_pad.transpose(1, 0, 2).reshape(D_IN, ntab), BF16)

    cfg = Cfg(n=n, n_cores=n_cores, npc=npc, wpc=wpc, npad=npad, th=th)

    # slot mapping: pair pr, half h, window-in-pair wl, j:
    #   slot = pr*4*ni + h*2*ni + wl*ni + j
    CPK = T2 * 40 + 4 * ni16
    in_maps = []
    for k in range(n_cores):
        m, rsrc, hbit, g2 = percore[k]
        ecnt = int(m.sum())
        cnt2 = cnt2_all[k]
        starts = np.zeros(wpc * 2, np.int64)
        starts[1:] = np.cumsum(cnt2)[:-1]
        order = np.argsort(g2, kind="stable")
        j_within = np.arange(ecnt) - starts[g2[order]]
        wo = g2[order] // 2
        ho = g2[order] % 2
        dest = (wo // 2) * (4 * ni) + ho * (2 * ni) + (wo % 2) * ni + j_within

        A_ = np.zeros((flat, 4), F32)
        A_[dest] = edge_attr[m][order]
        IDX = np.zeros(flat, np.int16)   # pad slots gather row 0 (harmless)
        IDX[dest] = (rsrc[order] - hbit[order] * half).astype(np.int16)
        L_ = np.zeros(flat, F32)
        L_[dest] = ldst[m][order]
        E_ = np.zeros((flat, FC_IN), F32)
        E_[dest] = ele[m][order]

        # pack per pair [P, CPK] bf16-bits:
        #   [0, 24*T2)        e1 (3 comps) each replicated x8, per tile
        #   [24*T2, 32*T2)    e0 replicated x8, per tile
        #   [32*T2, 40*T2)    ldst replicated x8, per tile
        #   [40*T2, +4*ni16)  gather idx int16 bits (half0 pair-list, half1)
        pk = np.zeros((npairs, P, CPK), np.uint16)

        def tileize(v, c):  # [flat, c] -> [npairs, P, T2*c]
            return (v.reshape(npairs, T2, P, c).transpose(0, 2, 1, 3)
                    .reshape(npairs, P, T2 * c))

        a8 = np.zeros((flat, 3, 8), F32)
        a8[dest] = np.repeat(edge_attr[m][order][:, 1:4][:, :, None], 8, axis=2)
        pk[:, :, 0:24 * T2] = tileize(
            a8.reshape(flat, 24).astype(BF16), 24).view(np.uint16)
        e0r = np.repeat(A_[:, 0:1], 8, axis=1).astype(BF16)
        pk[:, :, 24 * T2:32 * T2] = tileize(e0r, 8).view(np.uint16)
        l8 = np.repeat(L_[:, None], 8, axis=1).astype(BF16)
        pk[:, :, 32 * T2:40 * T2] = tileize(l8, 8).view(np.uint16)
        # idx: per pair per half a 2*ni list (w0 then w1; w1 slots of a
        # single-window tail pair stay -1), wrapped j -> (j%16, j//16),
        # replicated over the 8 16-partition channel groups
        IW = IDX.reshape(npairs, 2, 2 * ni16, 16).swapaxes(2, 3)
        IW8 = np.broadcast_to(IW[:, :, None, :, :],
                              (npairs, 2, 8, 16, 2 * ni16))
        pk[:, :, 40 * T2:40 * T2 + 2 * ni16] = (
            IW8[:, 0].reshape(npairs, P, 2 * ni16).view(np.uint16))
        pk[:, :, 40 * T2 + 2 * ni16:40 * T2 + 4 * ni16] = (
            IW8[:, 1].reshape(npairs, P, 2 * ni16).view(np.uint16))

        eleT = np.ascontiguousarray(E_.reshape(ep, FC_IN).T, BF16)

        xtb_k = np.roll(xTbf_global, -k * npad, axis=1)

        in_maps.append({
            "xTbf": np.ascontiguousarray(xtb_k),
            "Wb": Wb,
            "pack": pk.view(BF16),
            "eleT": eleT,
            "fcW1": np.ascontiguousarray(fc_W1, BF16),
            "fcW2": np.ascontiguousarray(fc_W2, BF16),
            "Wl2_0c": np.ascontiguousarray(W_l2_0, BF16),
            "Wl2_1c": np.ascontiguousarray(W_l2_1, BF16),
        })
    return cfg, in_maps, node_attr


# ---------------------------------------------------------------- device program

_PROG_CACHE = {}


def _build(cfg: Cfg):
    if cfg in _PROG_CACHE:
        return _PROG_CACHE[cfg]

    th, wpc, npad = cfg.th, cfg.wpc, cfg.npad
    tt = 2 * th
    T2 = 2 * tt
    ni = th * P
    ni16 = ni // 16
    npairs = (wpc + 1) // 2
    ep = npairs * T2 * P
    ntab = cfg.n_cores * npad
    half = ntab // 2
    gwc = cfg.n_cores * wpc
    CPK = T2 * 40 + 4 * ni16
    bf = mybir.dt.bfloat16
    f32 = mybir.dt.float32
    i16 = mybir.dt.int16

    nc = bacc.Bacc("TRN2", target_bir_lowering=False, debug=False,
                   num_devices=cfg.n_cores, num_swdge_queues=2)

    xTbf = nc.dram_tensor("xTbf", [D_IN, ntab], bf, kind="ExternalInput")
    Wb_d = nc.dram_tensor("Wb", [D_IN, 320], bf, kind="ExternalInput")
    pack_d = nc.dram_tensor("pack", [npairs, P, CPK], bf, kind="ExternalInput")
    eleT = nc.dram_tensor("eleT", [FC_IN, ep], bf, kind="ExternalInput")
    fcW1 = nc.dram_tensor("fcW1", [FC_IN, FC_H], bf, kind="ExternalInput")
    fcW2 = nc.dram_tensor("fcW2", [FC_H, WN], bf, kind="ExternalInput")
    Wl2_0c = nc.dram_tensor("Wl2_0c", [96, MUL0], bf, kind="ExternalInput")
    Wl2_1c = nc.dram_tensor("Wl2_1c", [96, MUL1], bf, kind="ExternalInput")
    out_d = nc.dram_tensor("out", [npad, D_IN], f32, kind="ExternalOutput")
    y_table = nc.dram_tensor("y_table", [ntab, DE], bf)

    if cfg.null:
        with tile.TileContext(nc) as tc:
            with tc.tile_pool(name="nullp", bufs=1) as npool:
                tnull = npool.tile([P, D_IN], f32)
                nc.gpsimd.memset(tnull[:], 0.0)
                nc.sync.dma_start(out=tnull[:, 0:4], in_=pack_d[0, :, 0:4])
                nc.vector.tensor_scalar(out=tnull[:], in0=tnull[:], scalar1=0.0,
                                        scalar2=None, op0=mybir.AluOpType.mult)
                nc.sync.dma_start(out=out_d[0:P, :], in_=tnull[:])
        nc.compile()
        _PROG_CACHE[cfg] = nc
        return nc

    MU = mybir.AluOpType.mult
    AD = mybir.AluOpType.add
    EQ = mybir.AluOpType.is_equal
    CP = mybir.ActivationFunctionType.Copy
    GB = 8  # build-loop window batch

    with tile.TileContext(nc) as tc:
        with (
            tc.tile_pool(name="const", bufs=1) as cpool,
            tc.tile_pool(name="work", bufs=2) as wp,
            tc.tile_pool(name="we", bufs=2) as we,
            tc.tile_pool(name="msgp", bufs=2) as mp,
        ):
            # ---- constants
            iota_i = cpool.tile([P, P], mybir.dt.int32)
            nc.gpsimd.iota(iota_i[:], pattern=[[1, P]], base=0, channel_multiplier=0)
            iota_bf = cpool.tile([P, P], bf)
            nc.vector.tensor_copy(out=iota_bf[:], in_=iota_i[:])
            ident = cpool.tile([P, P], bf)
            make_identity(nc, ident[:])

            fcW1_sb = cpool.tile([FC_IN, FC_H], bf)
            nc.sync.dma_start(out=fcW1_sb[:], in_=fcW1[:, :])
            fcW2_sb = cpool.tile([FC_H, WN], bf)
            nc.sync.dma_start(out=fcW2_sb[:], in_=fcW2[:, :])
            Wl20_sb = cpool.tile([96, MUL0], bf)
            nc.sync.dma_start(out=Wl20_sb[:], in_=Wl2_0c[:, :])
            Wl21_sb = cpool.tile([96, MUL1], bf)
            nc.sync.dma_start(out=Wl21_sb[:], in_=Wl2_1c[:, :])
            Wb0 = cpool.tile([P, 320], bf)
            nc.sync.dma_start(out=Wb0[:], in_=Wb_d[0:P, :])
            Wb1 = cpool.tile([D_IN - P, 320], bf)
            nc.sync.dma_start(out=Wb1[:], in_=Wb_d[P:D_IN, :])

            s_store = cpool.tile([P, wpc * D_IN], f32)

            for _rep in range(cfg.repeat):
                # ---- build phase: y table (+ s for the local first wpc windows)
                with tc.tile_pool(name="psA", bufs=2, space="PSUM") as psA:
                    for g0 in range(0, gwc, GB):
                        gb = min(GB, gwc - g0)
                        xab = wp.tile([P, GB * P], bf, tag="xab")
                        nc.sync.dma_start(out=xab[:, 0:gb * P],
                                          in_=xTbf[0:P, g0 * P:(g0 + gb) * P])
                        xbb = wp.tile([D_IN - P, GB * P], bf, tag="xbb")
                        nc.sync.dma_start(out=xbb[:, 0:gb * P],
                                          in_=xTbf[P:D_IN, g0 * P:(g0 + gb) * P])
                        ybuf = wp.tile([P, GB * DE], bf, tag="ybuf")
                        if cfg.ab_no_build:
                            if g0 == 0:
                                nc.scalar.activation(
                                    out=s_store[:, 0:16],
                                    in_=iota_bf[:, 0:16], func=CP)
                                nc.vector.tensor_copy(out=ybuf[:, 0:16],
                                                      in_=iota_bf[:, 0:16])
                            continue
                        if cfg.simsafe:
                            nc.gpsimd.memset(
                                ybuf[:].rearrange("p (g f) -> p g f", f=DE)
                                [:, :, D_IN:DE], 0.0)
                        j = 0
                        while j < gb:
                            g = g0 + j
                            if g >= wpc and j + 1 < gb:
                                # two non-local windows share one PSUM tile
                                # and one f32->bf16 copy
                                yp2 = psA.tile([P, 2 * D_IN], f32, tag="yp")
                                for u in range(2):
                                    nc.tensor.matmul(
                                        out=yp2[:, u * D_IN:(u + 1) * D_IN],
                                        lhsT=xab[:, (j + u) * P:(j + u + 1) * P],
                                        rhs=Wb0[:, 0:D_IN],
                                        start=True, stop=False)
                                    nc.tensor.matmul(
                                        out=yp2[:, u * D_IN:(u + 1) * D_IN],
                                        lhsT=xbb[:, (j + u) * P:(j + u + 1) * P],
                                        rhs=Wb1[:, 0:D_IN],
                                        start=False, stop=True)
                                ydst = ybuf[:].rearrange(
                                    "p (g f) -> p g f", f=DE)[:, j:j + 2, 0:D_IN]
                                ysrc = yp2[:].rearrange(
                                    "p (g f) -> p g f", f=D_IN)
                                if (g // 2) % 2 == 0:
                                    nc.vector.tensor_copy(out=ydst, in_=ysrc)
                                else:
                                    nc.scalar.activation(out=ydst, in_=ysrc,
                                                         func=CP)
                                j += 2
                                continue
                            ncols = 320 if g < wpc else D_IN
                            yp = psA.tile([P, 320], f32, tag="yp")
                            nc.tensor.matmul(out=yp[:, 0:ncols],
                                             lhsT=xab[:, j * P:(j + 1) * P],
                                             rhs=Wb0[:, 0:ncols],
                                             start=True, stop=False)
                            nc.tensor.matmul(out=yp[:, 0:ncols],
                                             lhsT=xbb[:, j * P:(j + 1) * P],
                                             rhs=Wb1[:, 0:ncols],
                                             start=False, stop=True)
                            ydst = ybuf[:, j * DE:j * DE + D_IN]
                            if j % 2 == 0:
                                nc.vector.tensor_copy(out=ydst, in_=yp[:, 0:D_IN])
                            else:
                                nc.scalar.activation(out=ydst, in_=yp[:, 0:D_IN],
                                                     func=CP)
                            if g < wpc:
                                nc.scalar.activation(
                                    out=s_store[:, g * D_IN:(g + 1) * D_IN],
                                    in_=yp[:, D_IN:320], func=CP)
                            j += 1
                        nc.sync.dma_start(
                            out=y_table[g0 * P:(g0 + gb) * P, :]
                                .rearrange("(g p) f -> p g f", p=P),
                            in_=ybuf[:, 0:gb * DE]
                                .rearrange("p (g f) -> p g f", f=DE))

                # ---- edge + node phases, one window PAIR at a time
                with (
                    tc.tile_pool(name="psE", bufs=2, space="PSUM") as psE,
                    tc.tile_pool(name="psZ", bufs=2, space="PSUM") as psZ,
                    tc.tile_pool(name="psN", bufs=2, space="PSUM") as psN,
                ):
                    for pr in range(npairs):
                        w0 = 2 * pr
                        ps = min(2, wpc - w0)
                        TT = T2
                        pk = we.tile([P, CPK], bf, tag="pk")
                        nc.sync.dma_start(out=pk[:], in_=pack_d[pr, :, :])
                        el_sb = we.tile([FC_IN, T2 * P], bf, tag="ele")
                        nc.sync.dma_start(
                            out=el_sb[:, 0:TT * P],
                            in_=eleT[:, pr * T2 * P:pr * T2 * P + TT * P])
                        a84 = pk[:, 0:24 * TT].rearrange(
                            "p (t c r) -> p t c r", c=3, r=8)
                        e0rv = pk[:, 24 * T2:24 * T2 + 8 * TT].rearrange(
                            "p (t r) -> p t r", r=8)
                        l8v = pk[:, 32 * T2:32 * T2 + 8 * TT].rearrange(
                            "p (t r) -> p t r", r=8)
                        idxv = pk[:, 40 * T2:40 * T2 + 4 * ni16].bitcast(i16)

                        # gathers: half0 pair-list then half1 (w1 slots of a
                        # tail pair carry idx=-1 -> skipped)
                        ys_all = we.tile([P, T2 * DE], bf, tag="ys")
                        if cfg.ab_no_gather:
                            nc.scalar.activation(out=ys_all[:, 0:16],
                                                 in_=iota_bf[:, 0:16], func=CP)
                        if not cfg.ab_no_gather:
                            nc.gpsimd.dma_gather(
                                out_ap=ys_all[:, 0:2 * th * DE].rearrange(
                                    "p (t f) -> p t f", f=DE),
                                in_ap=y_table[0:half, :],
                                idxs_ap=idxv[:, 0:2 * ni16],
                                num_idxs=2 * ni, num_idxs_reg=2 * ni,
                                elem_size=DE, single_packet=False)
                            nc.gpsimd.dma_gather(
                                out_ap=ys_all[:, 2 * th * DE:T2 * DE].rearrange(
                                    "p (t f) -> p t f", f=DE),
                                in_ap=y_table[half:ntab, :],
                                idxs_ap=idxv[:, 2 * ni16:4 * ni16],
                                num_idxs=2 * ni, num_idxs_reg=2 * ni,
                                elem_size=DE, single_packet=False,
                                queue_num=0 if cfg.sim_1q else 1)

                        # FC net -> per-edge tp weights w4_sb [P, TT*WN]
                        w4_sb = mp.tile([P, T2 * WN], bf, tag="w4")
                        if cfg.ab_no_fc:
                            nc.scalar.activation(out=w4_sb[:, 0:16],
                                                 in_=iota_bf[:, 0:16], func=CP)
                        for f0 in ([] if cfg.ab_no_fc else range(0, TT, 4)):
                            fs = min(4, TT - f0)
                            hT_ps = psE.tile([FC_H, 4 * P], f32, tag="ht",
                                             bufs=1)
                            nc.tensor.matmul(
                                out=hT_ps[:, 0:fs * P], lhsT=fcW1_sb[:],
                                rhs=el_sb[:, f0 * P:(f0 + fs) * P],
                                start=True, stop=True)
                            hT_sb = mp.tile([FC_H, 4 * P], bf, tag="hts")
                            nc.scalar.activation(
                                out=hT_sb[:, 0:fs * P], in_=hT_ps[:, 0:fs * P],
                                func=(mybir.ActivationFunctionType.Sigmoid
                                      if cfg.dbg_sigmoid else
                                      mybir.ActivationFunctionType.Silu))
                            for p0 in range(0, fs, 2):
                                w2_ps = psE.tile([P, 2 * WN], f32, tag="w")
                                for j in range(2):
                                    nc.tensor.matmul(
                                        out=w2_ps[:, j * WN:(j + 1) * WN],
                                        lhsT=hT_sb[:, (p0 + j) * P:(p0 + j + 1) * P],
                                        rhs=fcW2_sb[:], start=True, stop=True)
                                wdst = w4_sb[:, (f0 + p0) * WN:(f0 + p0 + 2) * WN]
                                nc.scalar.activation(out=wdst, in_=w2_ps[:],
                                                     func=CP)

                        # selection matrices + message build, chunked in
                        # tile-halves so sel matmuls overlap later DVE work
                        S4 = mp.tile([P, T2 * P], bf, tag="S")
                        msg4 = mp.tile([P, T2 * D_MID], bf, tag="msg")
                        tw = mp.tile([P, T2 * 2 * MUL0], bf, tag="tw")
                        dm = mp.tile([P, T2 * 96], bf, tag="dm")
                        ds = mp.tile([P, T2 * MUL1], bf, tag="ds")
                        t3t = mp.tile([P, T2 * MUL1], bf, tag="t3")
                        if cfg.ab_no_msg:
                            for tl in (msg4, tw, dm, ds, t3t):
                                nc.scalar.activation(out=tl[:, 0:16],
                                                     in_=iota_bf[:, 0:16],
                                                     func=CP)
                        if cfg.ab_no_s4:
                            nc.scalar.activation(out=S4[:, 0:16],
                                                 in_=iota_bf[:, 0:16], func=CP)
                        vtt = ((lambda **kw: None) if cfg.ab_no_msg
                               else nc.vector.tensor_tensor)
                        CH = 2 * th
                        for c0 in range(0, T2, CH):
                            TT = CH
                            a84 = pk[:, 24 * c0:24 * (c0 + CH)].rearrange(
                                "p (t c r) -> p t c r", c=3, r=8)
                            e0rv = pk[:, 24 * T2 + 8 * c0:
                                      24 * T2 + 8 * (c0 + CH)].rearrange(
                                "p (t r) -> p t r", r=8)
                            l8v = pk[:, 32 * T2 + 8 * c0:
                                     32 * T2 + 8 * (c0 + CH)].rearrange(
                                "p (t r) -> p t r", r=8)
                            if not cfg.ab_no_s4:
                                nc.vector.tensor_tensor(
                                    out=S4[:, c0 * P:(c0 + CH) * P].rearrange(
                                        "p (t v r) -> p t v r", v=16, r=8),
                                    in0=iota_bf[:].rearrange(
                                        "p (v r) -> p v r", r=8)
                                        .unsqueeze(1).broadcast_to(
                                        [P, TT, 16, 8]),
                                    in1=l8v.unsqueeze(2).broadcast_to(
                                        [P, TT, 16, 8]),
                                    op=EQ)
                            ysv = ys_all[:, c0 * DE:(c0 + CH) * DE].rearrange(
                                "p (t f) -> p t f", f=DE)
                            ys0v = ysv[:, :, 0:MUL0]
                            ys1v = ysv[:, :, MUL0:D_IN].rearrange(
                                "p t (c u) -> p t c u", u=MUL1)
                            w4v = w4_sb[:, c0 * WN:(c0 + CH) * WN].rearrange(
                                "p (t k) -> p t k", k=WN)
                            msgv = msg4[:, c0 * D_MID:(c0 + CH) * D_MID]\
                                .rearrange("p (t k) -> p t k", k=D_MID)
                            msg1 = msgv[:, :, 96:D_MID].rearrange(
                                "p t (c x) -> p t c x", x=96)
                            twv = tw[:, c0 * 2 * MUL0:(c0 + CH) * 2 * MUL0]\
                                .rearrange("p (t r u) -> p t r u", r=2, u=MUL0)
                            vtt(out=twv,
                                in0=w4v[:, :, 0:2 * MUL0].rearrange(
                                    "p t (r u) -> p t r u", u=MUL0),
                                in1=ys0v.unsqueeze(2).broadcast_to(
                                    [P, TT, 2, MUL0]),
                                op=MU)
                            vtt(out=msgv[:, :, 0:MUL0].rearrange(
                                    "p t (v r) -> p t v r", r=8),
                                in0=twv[:, :, 0, :].rearrange(
                                    "p t (v r) -> p t v r", r=8),
                                in1=e0rv.unsqueeze(2).broadcast_to(
                                    [P, TT, MUL0 // 8, 8]),
                                op=MU)
                            dmv = dm[:, c0 * 96:(c0 + CH) * 96].rearrange(
                                "p (t c u) -> p t c u", c=3, u=MUL1)
                            vtt(out=dmv.rearrange(
                                    "p t c (v r) -> p t c v r", r=8),
                                in0=ys1v.rearrange(
                                    "p t c (v r) -> p t c v r", r=8),
                                in1=a84.unsqueeze(3).broadcast_to(
                                    [P, TT, 3, MUL1 // 8, 8]),
                                op=MU)
                            dsv = ds[:, c0 * MUL1:(c0 + CH) * MUL1].rearrange(
                                "p (t u) -> p t u", u=MUL1)
                            vtt(out=dsv, in0=dmv[:, :, 0, :],
                                in1=dmv[:, :, 1, :], op=AD)
                            vtt(out=dsv, in0=dsv, in1=dmv[:, :, 2, :], op=AD)
                            vtt(out=msgv[:, :, MUL0:96], in0=dsv,
                                in1=w4v[:, :, 160:WN], op=MU)
                            t3v = t3t[:, c0 * MUL1:(c0 + CH) * MUL1].rearrange(
                                "p (t u) -> p t u", u=MUL1)
                            vtt(out=t3v.rearrange("p t (v r) -> p t v r", r=8),
                                in0=w4v[:, :, 128:160].rearrange(
                                    "p t (v r) -> p t v r", r=8),
                                in1=e0rv.unsqueeze(2).broadcast_to(
                                    [P, TT, MUL1 // 8, 8]),
                                op=MU)
                            vtt(out=msg1[:, :, :, 0:MUL0].rearrange(
                                    "p t c (v r) -> p t c v r", r=8),
                                in0=twv[:, :, 1:2, :].broadcast_to(
                                    [P, TT, 3, MUL0]).rearrange(
                                    "p t c (v r) -> p t c v r", r=8),
                                in1=a84.unsqueeze(3).broadcast_to(
                                    [P, TT, 3, MUL0 // 8, 8]),
                                op=MU)
                            vtt(out=msg1[:, :, :, MUL0:96],
                                in0=t3v.unsqueeze(2).broadcast_to(
                                    [P, TT, 3, MUL1]),
                                in1=ys1v, op=MU)

                        # scatter: one selection matmul per tile
                        # (lhsT = one-hot S4 tile; z[wl] accumulates in its
                        # own PSUM bank, one group per bank)
                        zz = [psZ.tile([P, D_MID], f32, tag=f"zt{wl}",
                                       name=f"z_{wl}", bufs=1)
                              for wl in range(2)]
                        for wl in range(2):
                            tiles = ([wl * th + t for t in range(th)]
                                     + [(2 + wl) * th + t for t in range(th)])
                            if cfg.ab_no_sel:
                                tiles = tiles[:1]
                            for i, t in enumerate(tiles):
                                nc.tensor.matmul(
                                    out=zz[wl][:],
                                    lhsT=S4[:, t * P:(t + 1) * P],
                                    rhs=msg4[:, t * D_MID:(t + 1) * D_MID],
                                    start=(i == 0),
                                    stop=(i == len(tiles) - 1))

                        # ---- node phase (per window in the pair)
                        out_sb = mp.tile([P, 2 * D_IN], f32, tag="outsb")
                        for wl in ([] if cfg.ab_no_node else range(ps)):
                            w = w0 + wl
                            z_sb = mp.tile([P, D_MID], bf, tag="zsb")
                            nc.scalar.activation(out=z_sb[:], in_=zz[wl][:],
                                                 func=CP)
                            o_ps = psN.tile([P, D_IN], f32, tag="o", bufs=1)
                            for b in range(4):
                                zT_ps = psZ.tile([96, P], bf, tag=f"zt{wl}",
                                                 name=f"zT_{wl}", bufs=1)
                                nc.tensor.transpose(
                                    out=zT_ps[:],
                                    in_=z_sb[:, b * 96:(b + 1) * 96],
                                    identity=ident[:])
                                zT_sb = mp.tile([96, P], bf, tag="zts")
                                if b % 2 == 0:
                                    nc.scalar.activation(out=zT_sb[:],
                                                         in_=zT_ps[:], func=CP)
                                else:
                                    nc.vector.tensor_copy(out=zT_sb[:],
                                                          in_=zT_ps[:])
                                if b == 0:
                                    nc.tensor.matmul(out=o_ps[:, 0:MUL0],
                                                     lhsT=zT_sb[:],
                                                     rhs=Wl20_sb[:],
                                                     start=True, stop=True)
                                else:
                                    c = b - 1
                                    nc.tensor.matmul(
                                        out=o_ps[:, MUL0 + c * MUL1:
                                                 MUL0 + (c + 1) * MUL1],
                                        lhsT=zT_sb[:], rhs=Wl21_sb[:],
                                        start=True, stop=True)
                            nc.vector.tensor_tensor(
                                out=out_sb[:, wl * D_IN:(wl + 1) * D_IN],
                                in0=o_ps[:],
                                in1=s_store[:, w * D_IN:(w + 1) * D_IN], op=AD)
                        if cfg.ab_no_node:
                            nc.scalar.activation(out=out_sb[:, 0:16],
                                                 in_=iota_bf[:, 0:16], func=CP)
                        nc.sync.dma_start(
                            out=out_d[w0 * P:(w0 + ps) * P, :]
                                .rearrange("(g p) f -> p g f", p=P),
                            in_=out_sb[:, 0:ps * D_IN]
                                .rearrange("p (g f) -> p g f", f=D_IN))

    nc.compile()
    _PROG_CACHE[cfg] = nc
    return nc


# ---------------------------------------------------------------- entry point

def _assemble(cfg: Cfg, results, node_attr):
    outs = [results[k]["out"][:cfg.npc] for k in range(cfg.n_cores)]
    o = np.concatenate(outs, axis=0).astype(F32)
    o = np.concatenate([o[:, :MUL0], _from_cmajor(o[:, MUL0:])], axis=1)
    return o * node_attr


def kernel(**inputs):
    cfg, in_maps, node_attr = _prep(inputs, n_cores=8)
    nc = _build(cfg)
    res = run_bass_kernel_spmd(nc, in_maps, core_ids=list(range(cfg.n_cores)))
    return _assemble(cfg, res.results, node_attr)


# revision 31
# speedup vs baseline: 3.2282x; 3.2282x over previous
"""Trainium2 Bass kernel for the e3nn-style GNN convolution layer.

kernel(**inputs) takes FULL (unsharded) numpy inputs and returns the FULL
[N, 160] float32 output.  Internally shards edges across 8 NeuronCores by
destination-node range, runs one SPMD Bass program, and reassembles on host.

Design (v2):
  host prep   - fold all scalar normalizations into weights; x~ = node_input *
                node_attr; per-core node table ROTATED so the core's own slice
                comes first (gather indices remapped accordingly); edges
                grouped by (dst-window-PAIR, src-table-half, window) padded to
                th 128-edge tiles per (half, window); ALL per-pair edge
                operands packed into ONE bf16 tensor (e1 x8, e0 x8, ldst x8,
                gather idx bits).
  build phase - per 8 global windows: 2 batched xTbf loads, per window a
                fused bf16 matmul (n=320 for the first wpc windows: lin1 y +
                self-connection s; n=160 otherwise), y copied bf16 into an
                8-window staging tile, ONE batched DMA store to the y_table.
  edge phase  - per window PAIR: 1 pack DMA + 1 eleT DMA + 2 dma_gathers
                (2 swdge queues, 2ni idxs each); FC net (bf16 matmuls + Silu);
                message build on DVE/Pool all in 2x perf mode; selection
                matmuls producing TRANSPOSED z blocks directly in PSUM.
  node phase  - per window: one zT copy (ACT), lin2 (4 bf16 matmuls), add s,
                batched output store per pair.
"""

import math
from dataclasses import dataclass

import numpy as np
import ml_dtypes

import concourse.bacc as bacc
import concourse.mybir as mybir
import concourse.tile as tile
from concourse.bass_utils import run_bass_kernel_spmd
from concourse.masks import make_identity

BF16 = ml_dtypes.bfloat16
F32 = np.float32

MUL0 = 64
MUL1 = 32
FC_IN = 16
FC_H = 64
WN = 2 * MUL0 + 2 * MUL1  # 192 per-edge tp weights
D_IN = MUL0 + 3 * MUL1    # 160
DE = 256                  # padded y-table row elems (512 B in bf16)
D_MID = 4 * (MUL0 + MUL1) # 384 = [z0 (96) | z1_c0 | z1_c1 | z1_c2]
NUM_NEIGH = 10.0
C_S = math.sin(math.pi / 8.0)
C_X = math.cos(math.pi / 8.0)
P = 128


@dataclass(frozen=True)
class Cfg:
    n: int
    n_cores: int
    npc: int          # nodes per core
    wpc: int          # 128-node windows per core
    npad: int         # wpc * 128
    th: int           # tiles per (window, table-half)
    repeat: int = 1
    null: bool = False
    simsafe: bool = False  # memset staging pads (CoreSim exec strictness)
    dbg_sigmoid: bool = False  # Sigmoid instead of Silu (CoreSim exec debug)
    sim_1q: bool = False   # force both gathers onto swdge queue 0 (sim only)
    # timing ablations (break correctness)
    ab_no_build: bool = False
    ab_no_gather: bool = False
    ab_no_fc: bool = False
    ab_no_s4: bool = False
    ab_no_msg: bool = False
    ab_no_sel: bool = False
    ab_no_node: bool = False


def _to_cmajor(x_uc):
    s = x_uc.shape[:-1]
    return x_uc.reshape(*s, MUL1, 3).swapaxes(-1, -2).reshape(*s, 96)


def _from_cmajor(x_cu):
    s = x_cu.shape[:-1]
    return x_cu.reshape(*s, 3, MUL1).swapaxes(-1, -2).reshape(*s, 96)


# ---------------------------------------------------------------- host prep

def _prep(inputs, n_cores=8):
    node_input = np.asarray(inputs["node_input"], F32)
    node_attr = np.asarray(inputs["node_attr"], F32)
    edge_src = np.asarray(inputs["edge_src"]).astype(np.int64)
    edge_dst = np.asarray(inputs["edge_dst"]).astype(np.int64)
    edge_attr = np.asarray(inputs["edge_attr"], F32)
    ele = np.asarray(inputs["edge_length_embedded"], F32)

    n = node_input.shape[0]
    assert n % n_cores == 0
    npc = n // n_cores
    wpc = (npc + P - 1) // P
    npad = wpc * P
    ntab = n_cores * npad
    half = ntab // 2
    assert half <= 32767 and half % npad == 0

    inv0 = 1.0 / math.sqrt(MUL0)
    inv1 = 1.0 / math.sqrt(MUL1)
    invm = 1.0 / math.sqrt(MUL0 + MUL1)
    invnb = 1.0 / math.sqrt(NUM_NEIGH)

    x = node_input * node_attr
    xT = np.concatenate([x[:, :MUL0], _to_cmajor(x[:, MUL0:])], axis=1).T
    xT = np.ascontiguousarray(xT, F32)   # [160, n]

    W_sc0 = np.asarray(inputs["W_sc0"], F32) * (inv0 * C_S)
    W_sc1 = np.asarray(inputs["W_sc1"], F32) * (inv1 * C_S)
    W_l1_0 = np.asarray(inputs["W_l1_0"], F32) * inv0
    W_l1_1 = np.asarray(inputs["W_l1_1"], F32) * inv1
    fc_W1 = np.asarray(inputs["fc_W1"], F32) * (1.0 / math.sqrt(FC_IN))
    fc_W2 = np.asarray(inputs["fc_W2"], F32) * (1.0 / math.sqrt(FC_H))
    obase = invm * C_X * invnb
    W_l2_0 = np.asarray(inputs["W_l2_0"], F32) * obase
    W_l2_0 = W_l2_0.copy()
    W_l2_0[MUL0:, :] *= 1.0 / math.sqrt(3.0)
    W_l2_1 = np.asarray(inputs["W_l2_1"], F32) * obase

    def blockdiag(*ms):
        rows = sum(m.shape[0] for m in ms)
        cols = sum(m.shape[1] for m in ms)
        out = np.zeros((rows, cols), F32)
        r = c = 0
        for m in ms:
            out[r:r + m.shape[0], c:c + m.shape[1]] = m
            r += m.shape[0]
            c += m.shape[1]
        return out

    Wsc_big = blockdiag(W_sc0, W_sc1, W_sc1, W_sc1)
    Wl1_big = blockdiag(W_l1_0, W_l1_1, W_l1_1, W_l1_1)
    # build-loop rhs: [Wl1 (160) | Wsc (160)] — s half read for local windows
    Wb = np.ascontiguousarray(np.concatenate([Wl1_big, Wsc_big], axis=1), BF16)

    # ---- edge sharding
    core = edge_dst // npc
    local = edge_dst - core * npc
    win = local // P
    ldst = (local - win * P).astype(F32)
    src_remap = (edge_src // npc) * npad + (edge_src % npc)

    # per-core rotated table: core k's table row = (global_remap - k*npad) % ntab
    cnt2_all = np.zeros((n_cores, wpc * 2), np.int64)
    percore = []
    for k in range(n_cores):
        m = core == k
        rsrc = (src_remap[m] - k * npad) % ntab
        hbit = (rsrc >= half).astype(np.int64)
        g2 = win[m] * 2 + hbit
        percore.append((m, rsrc, hbit, g2))
        cnt2_all[k] = np.bincount(g2, minlength=wpc * 2)
    th = max(1, int((cnt2_all.max() + P - 1) // P))
    ni = th * P
    ni16 = ni // 16
    tt = 2 * th            # tiles per (window)
    T2 = 2 * tt            # tiles per full pair
    npairs = (wpc + 1) // 2
    flat = npairs * T2 * P  # total slots (tail pair padded to a full pair)
    ep = flat

    xT_pad = np.zeros((n_cores, D_IN, npad), F32)
    for k in range(n_cores):
        xT_pad[k, :, :npc] = xT[:, k * npc:(k + 1) * npc]
    xTbf_global = np.ascontiguousarray(
        xT_pad.transpose(1, 0, 2).reshape(D_IN, ntab), BF16)

    cfg = Cfg(n=n, n_cores=n_cores, npc=npc, wpc=wpc, npad=npad, th=th)

    # slot mapping: pair pr, half h, window-in-pair wl, j:
    #   slot = pr*4*ni + h*2*ni + wl*ni + j
    CPK = T2 * 40 + 4 * ni16
    in_maps = []
    for k in range(n_cores):
        m, rsrc, hbit, g2 = percore[k]
        ecnt = int(m.sum())
        cnt2 = cnt2_all[k]
        starts = np.zeros(wpc * 2, np.int64)
        starts[1:] = np.cumsum(cnt2)[:-1]
        order = np.argsort(g2, kind="stable")
        j_within = np.arange(ecnt) - starts[g2[order]]
        wo = g2[order] // 2
        ho = g2[order] % 2
        dest = (wo // 2) * (4 * ni) + ho * (2 * ni) + (wo % 2) * ni + j_within

        A_ = np.zeros((flat, 4), F32)
        A_[dest] = edge_attr[m][order]
        IDX = np.zeros(flat, np.int16)   # pad slots gather row 0 (harmless)
        IDX[dest] = (rsrc[order] - hbit[order] * half).astype(np.int16)
        L_ = np.zeros(flat, F32)
        L_[dest] = ldst[m][order]
        E_ = np.zeros((flat, FC_IN), F32)
        E_[dest] = ele[m][order]

        # pack per pair [P, CPK] bf16-bits:
        #   [0, 24*T2)        e1 (3 comps) each replicated x8, per tile
        #   [24*T2, 32*T2)    e0 replicated x8, per tile
        #   [32*T2, 40*T2)    ldst replicated x8, per tile
        #   [40*T2, +4*ni16)  gather idx int16 bits (half0 pair-list, half1)
        pk = np.zeros((npairs, P, CPK), np.uint16)

        def tileize(v, c):  # [flat, c] -> [npairs, P, T2*c]
            return (v.reshape(npairs, T2, P, c).transpose(0, 2, 1, 3)
                    .reshape(npairs, P, T2 * c))

        a8 = np.zeros((flat, 3, 8), F32)
        a8[dest] = np.repeat(edge_attr[m][order][:, 1:4][:, :, None], 8, axis=2)
        pk[:, :, 0:24 * T2] = tileize(
            a8.reshape(flat, 24).astype(BF16), 24).view(np.uint16)
        e0r = np.repeat(A_[:, 0:1], 8, axis=1).astype(BF16)
        pk[:, :, 24 * T2:32 * T2] = tileize(e0r, 8).view(np.uint16)
        l8 = np.repeat(L_[:, None], 8, axis=1).astype(BF16)
        pk[:, :, 32 * T2:40 * T2] = tileize(l8, 8).view(np.uint16)
        # idx: per pair per half a 2*ni list (w0 then w1; w1 slots of a
        # single-window tail pair stay -1), wrapped j -> (j%16, j//16),
        # replicated over the 8 16-partition channel groups
        IW = IDX.reshape(npairs, 2, 2 * ni16, 16).swapaxes(2, 3)
        IW8 = np.broadcast_to(IW[:, :, None, :, :],
                              (npairs, 2, 8, 16, 2 * ni16))
        pk[:, :, 40 * T2:40 * T2 + 2 * ni16] = (
            IW8[:, 0].reshape(npairs, P, 2 * ni16).view(np.uint16))
        pk[:, :, 40 * T2 + 2 * ni16:40 * T2 + 4 * ni16] = (
            IW8[:, 1].reshape(npairs, P, 2 * ni16).view(np.uint16))

        eleT = np.ascontiguousarray(E_.reshape(ep, FC_IN).T, BF16)

        xtb_k = np.roll(xTbf_global, -k * npad, axis=1)

        in_maps.append({
            "xTbf": np.ascontiguousarray(xtb_k),
            "Wb": Wb,
            "pack": pk.view(BF16),
            "eleT": eleT,
            "fcW1": np.ascontiguousarray(fc_W1, BF16),
            "fcW2": np.ascontiguousarray(fc_W2, BF16),
            "Wl2_0c": np.ascontiguousarray(W_l2_0, BF16),
            "Wl2_1c": np.ascontiguousarray(W_l2_1, BF16),
        })
    return cfg, in_maps, node_attr


# ---------------------------------------------------------------- device program

_PROG_CACHE = {}


def _build(cfg: Cfg):
    if cfg in _PROG_CACHE:
        return _PROG_CACHE[cfg]

    th, wpc, npad = cfg.th, cfg.wpc, cfg.npad
    tt = 2 * th
    T2 = 2 * tt
    ni = th * P
    ni16 = ni // 16
    npairs = (wpc + 1) // 2
    ep = npairs * T2 * P
    ntab = cfg.n_cores * npad
    half = ntab // 2
    gwc = cfg.n_cores * wpc
    CPK = T2 * 40 + 4 * ni16
    bf = mybir.dt.bfloat16
    f32 = mybir.dt.float32
    i16 = mybir.dt.int16

    nc = bacc.Bacc("TRN2", target_bir_lowering=False, debug=False,
                   num_devices=cfg.n_cores, num_swdge_queues=2)

    xTbf = nc.dram_tensor("xTbf", [D_IN, ntab], bf, kind="ExternalInput")
    Wb_d = nc.dram_tensor("Wb", [D_IN, 320], bf, kind="ExternalInput")
    pack_d = nc.dram_tensor("pack", [npairs, P, CPK], bf, kind="ExternalInput")
    eleT = nc.dram_tensor("eleT", [FC_IN, ep], bf, kind="ExternalInput")
    fcW1 = nc.dram_tensor("fcW1", [FC_IN, FC_H], bf, kind="ExternalInput")
    fcW2 = nc.dram_tensor("fcW2", [FC_H, WN], bf, kind="ExternalInput")
    Wl2_0c = nc.dram_tensor("Wl2_0c", [96, MUL0], bf, kind="ExternalInput")
    Wl2_1c = nc.dram_tensor("Wl2_1c", [96, MUL1], bf, kind="ExternalInput")
    out_d = nc.dram_tensor("out", [npad, D_IN], f32, kind="ExternalOutput")
    y_table = nc.dram_tensor("y_table", [ntab, DE], bf)

    if cfg.null:
        with tile.TileContext(nc) as tc:
            with tc.tile_pool(name="nullp", bufs=1) as npool:
                tnull = npool.tile([P, D_IN], f32)
                nc.gpsimd.memset(tnull[:], 0.0)
                nc.sync.dma_start(out=tnull[:, 0:4], in_=pack_d[0, :, 0:4])
                nc.vector.tensor_scalar(out=tnull[:], in0=tnull[:], scalar1=0.0,
                                        scalar2=None, op0=mybir.AluOpType.mult)
                nc.sync.dma_start(out=out_d[0:P, :], in_=tnull[:])
        nc.compile()
        _PROG_CACHE[cfg] = nc
        return nc

    MU = mybir.AluOpType.mult
    AD = mybir.AluOpType.add
    EQ = mybir.AluOpType.is_equal
    CP = mybir.ActivationFunctionType.Copy
    GB = 8  # build-loop window batch

    with tile.TileContext(nc) as tc:
        with (
            tc.tile_pool(name="const", bufs=1) as cpool,
            tc.tile_pool(name="work", bufs=2) as wp,
            tc.tile_pool(name="we", bufs=2) as we,
            tc.tile_pool(name="msgp", bufs=2) as mp,
        ):
            # ---- constants
            iota_i = cpool.tile([P, P], mybir.dt.int32)
            nc.gpsimd.iota(iota_i[:], pattern=[[1, P]], base=0, channel_multiplier=0)
            iota_bf = cpool.tile([P, P], bf)
            nc.vector.tensor_copy(out=iota_bf[:], in_=iota_i[:])
            ident = cpool.tile([P, P], bf)
            make_identity(nc, ident[:])

            fcW1_sb = cpool.tile([FC_IN, FC_H], bf)
            nc.sync.dma_start(out=fcW1_sb[:], in_=fcW1[:, :])
            fcW2_sb = cpool.tile([FC_H, WN], bf)
            nc.sync.dma_start(out=fcW2_sb[:], in_=fcW2[:, :])
            Wl20_sb = cpool.tile([96, MUL0], bf)
            nc.sync.dma_start(out=Wl20_sb[:], in_=Wl2_0c[:, :])
            Wl21_sb = cpool.tile([96, MUL1], bf)
            nc.sync.dma_start(out=Wl21_sb[:], in_=Wl2_1c[:, :])
            Wb0 = cpool.tile([P, 320], bf)
            nc.sync.dma_start(out=Wb0[:], in_=Wb_d[0:P, :])
            Wb1 = cpool.tile([D_IN - P, 320], bf)
            nc.sync.dma_start(out=Wb1[:], in_=Wb_d[P:D_IN, :])

            s_store = cpool.tile([P, wpc * D_IN], f32)

            for _rep in range(cfg.repeat):
                # ---- build phase: y table (+ s for the local first wpc windows)
                with tc.tile_pool(name="psA", bufs=2, space="PSUM") as psA:
                    for g0 in range(0, gwc, GB):
                        gb = min(GB, gwc - g0)
                        xab = wp.tile([P, GB * P], bf, tag="xab")
                        nc.sync.dma_start(out=xab[:, 0:gb * P],
                                          in_=xTbf[0:P, g0 * P:(g0 + gb) * P])
                        xbb = wp.tile([D_IN - P, GB * P], bf, tag="xbb")
                        nc.sync.dma_start(out=xbb[:, 0:gb * P],
                                          in_=xTbf[P:D_IN, g0 * P:(g0 + gb) * P])
                        ybuf = wp.tile([P, GB * DE], bf, tag="ybuf")
                        if cfg.ab_no_build:
                            if g0 == 0:
                                nc.scalar.activation(
                                    out=s_store[:, 0:16],
                                    in_=iota_bf[:, 0:16], func=CP)
                                nc.vector.tensor_copy(out=ybuf[:, 0:16],
                                                      in_=iota_bf[:, 0:16])
                            continue
                        if cfg.simsafe:
                            nc.gpsimd.memset(
                                ybuf[:].rearrange("p (g f) -> p g f", f=DE)
                                [:, :, D_IN:DE], 0.0)
                        j = 0
                        while j < gb:
                            g = g0 + j
                            if g >= wpc and j + 1 < gb:
                                # two non-local windows share one PSUM tile
                                # and one f32->bf16 copy
                                yp2 = psA.tile([P, 2 * D_IN], f32, tag="yp")
                                for u in range(2):
                                    nc.tensor.matmul(
                                        out=yp2[:, u * D_IN:(u + 1) * D_IN],
                                        lhsT=xab[:, (j + u) * P:(j + u + 1) * P],
                                        rhs=Wb0[:, 0:D_IN],
                                        start=True, stop=False)
                                    nc.tensor.matmul(
                                        out=yp2[:, u * D_IN:(u + 1) * D_IN],
                                        lhsT=xbb[:, (j + u) * P:(j + u + 1) * P],
                                        rhs=Wb1[:, 0:D_IN],
                                        start=False, stop=True)
                                ydst = ybuf[:].rearrange(
                                    "p (g f) -> p g f", f=DE)[:, j:j + 2, 0:D_IN]
                                ysrc = yp2[:].rearrange(
                                    "p (g f) -> p g f", f=D_IN)
                                if (g // 2) % 2 == 0:
                                    nc.vector.tensor_copy(out=ydst, in_=ysrc)
                                else:
                                    nc.scalar.activation(out=ydst, in_=ysrc,
                                                         func=CP)
                                j += 2
                                continue
                            ncols = 320 if g < wpc else D_IN
                            yp = psA.tile([P, 320], f32, tag="yp")
                            nc.tensor.matmul(out=yp[:, 0:ncols],
                                             lhsT=xab[:, j * P:(j + 1) * P],
                                             rhs=Wb0[:, 0:ncols],
                                             start=True, stop=False)
                            nc.tensor.matmul(out=yp[:, 0:ncols],
                                             lhsT=xbb[:, j * P:(j + 1) * P],
                                             rhs=Wb1[:, 0:ncols],
                                             start=False, stop=True)
                            ydst = ybuf[:, j * DE:j * DE + D_IN]
                            if j % 2 == 0:
                                nc.vector.tensor_copy(out=ydst, in_=yp[:, 0:D_IN])
                            else:
                                nc.scalar.activation(out=ydst, in_=yp[:, 0:D_IN],
                                                     func=CP)
                            if g < wpc:
                                nc.scalar.activation(
                                    out=s_store[:, g * D_IN:(g + 1) * D_IN],
                                    in_=yp[:, D_IN:320], func=CP)
                            j += 1
                        nc.sync.dma_start(
                            out=y_table[g0 * P:(g0 + gb) * P, :]
                                .rearrange("(g p) f -> p g f", p=P),
                            in_=ybuf[:, 0:gb * DE]
                                .rearrange("p (g f) -> p g f", f=DE))

                # ---- edge + node phases, one window PAIR at a time
                with (
                    tc.tile_pool(name="psE", bufs=2, space="PSUM") as psE,
                    tc.tile_pool(name="psZ", bufs=2, space="PSUM") as psZ,
                    tc.tile_pool(name="psN", bufs=2, space="PSUM") as psN,
                ):
                    for pr in range(npairs):
                        w0 = 2 * pr
                        ps = min(2, wpc - w0)
                        TT = T2
                        pk = we.tile([P, CPK], bf, tag="pk")
                        nc.sync.dma_start(out=pk[:], in_=pack_d[pr, :, :])
                        el_sb = we.tile([FC_IN, T2 * P], bf, tag="ele")
                        nc.sync.dma_start(
                            out=el_sb[:, 0:TT * P],
                            in_=eleT[:, pr * T2 * P:pr * T2 * P + TT * P])
                        a84 = pk[:, 0:24 * TT].rearrange(
                            "p (t c r) -> p t c r", c=3, r=8)
                        e0rv = pk[:, 24 * T2:24 * T2 + 8 * TT].rearrange(
                            "p (t r) -> p t r", r=8)
                        l8v = pk[:, 32 * T2:32 * T2 + 8 * TT].rearrange(
                            "p (t r) -> p t r", r=8)
                        idxv = pk[:, 40 * T2:40 * T2 + 4 * ni16].bitcast(i16)

                        # gathers: half0 pair-list then half1 (w1 slots of a
                        # tail pair carry idx=-1 -> skipped)
                        ys_all = we.tile([P, T2 * DE], bf, tag="ys")
                        if cfg.ab_no_gather:
                            nc.scalar.activation(out=ys_all[:, 0:16],
                                                 in_=iota_bf[:, 0:16], func=CP)
                        if not cfg.ab_no_gather:
                            nc.gpsimd.dma_gather(
                                out_ap=ys_all[:, 0:2 * th * DE].rearrange(
                                    "p (t f) -> p t f", f=DE),
                                in_ap=y_table[0:half, :],
                                idxs_ap=idxv[:, 0:2 * ni16],
                                num_idxs=2 * ni, num_idxs_reg=2 * ni,
                                elem_size=DE, single_packet=False)
                            nc.gpsimd.dma_gather(
                                out_ap=ys_all[:, 2 * th * DE:T2 * DE].rearrange(
                                    "p (t f) -> p t f", f=DE),
                                in_ap=y_table[half:ntab, :],
                                idxs_ap=idxv[:, 2 * ni16:4 * ni16],
                                num_idxs=2 * ni, num_idxs_reg=2 * ni,
                                elem_size=DE, single_packet=False,
                                queue_num=0 if cfg.sim_1q else 1)

                        # FC net -> per-edge tp weights w4_sb [P, TT*WN]
                        w4_sb = mp.tile([P, T2 * WN], bf, tag="w4")
                        if cfg.ab_no_fc:
                            nc.scalar.activation(out=w4_sb[:, 0:16],
                                                 in_=iota_bf[:, 0:16], func=CP)
                        for f0 in ([] if cfg.ab_no_fc else range(0, TT, 4)):
                            fs = min(4, TT - f0)
                            hT_ps = psE.tile([FC_H, 4 * P], f32, tag="ht",
                                             bufs=1)
                            nc.tensor.matmul(
                                out=hT_ps[:, 0:fs * P], lhsT=fcW1_sb[:],
                                rhs=el_sb[:, f0 * P:(f0 + fs) * P],
                                start=True, stop=True)
                            hT_sb = mp.tile([FC_H, 4 * P], bf, tag="hts")
                            nc.scalar.activation(
                                out=hT_sb[:, 0:fs * P], in_=hT_ps[:, 0:fs * P],
                                func=(mybir.ActivationFunctionType.Sigmoid
                                      if cfg.dbg_sigmoid else
                                      mybir.ActivationFunctionType.Silu))
                            for p0 in range(0, fs, 2):
                                w2_ps = psE.tile([P, 2 * WN], f32, tag="w")
                                for j in range(2):
                                    nc.tensor.matmul(
                                        out=w2_ps[:, j * WN:(j + 1) * WN],
                                        lhsT=hT_sb[:, (p0 + j) * P:(p0 + j + 1) * P],
                                        rhs=fcW2_sb[:], start=True, stop=True)
                                wdst = w4_sb[:, (f0 + p0) * WN:(f0 + p0 + 2) * WN]
                                nc.scalar.activation(out=wdst, in_=w2_ps[:],
                                                     func=CP)

                        # selection matrices + message build, chunked in
                        # tile-halves so sel matmuls overlap later DVE work
                        S4 = mp.tile([P, T2 * P], bf, tag="S")
                        msg4 = mp.tile([P, T2 * D_MID], bf, tag="msg")
                        tw = mp.tile([P, T2 * 2 * MUL0], bf, tag="tw")
                        dm = mp.tile([P, T2 * 96], bf, tag="dm")
                        ds = mp.tile([P, T2 * MUL1], bf, tag="ds")
                        t3t = mp.tile([P, T2 * MUL1], bf, tag="t3")
                        if cfg.ab_no_msg:
                            for tl in (msg4, tw, dm, ds, t3t):
                                nc.scalar.activation(out=tl[:, 0:16],
                                                     in_=iota_bf[:, 0:16],
                                                     func=CP)
                        if cfg.ab_no_s4:
                            nc.scalar.activation(out=S4[:, 0:16],
                                                 in_=iota_bf[:, 0:16], func=CP)
                        vtt = ((lambda **kw: None) if cfg.ab_no_msg
                               else nc.vector.tensor_tensor)
                        CH = 2 * th
                        for c0 in range(0, T2, CH):
                            TT = CH
                            a84 = pk[:, 24 * c0:24 * (c0 + CH)].rearrange(
                                "p (t c r) -> p t c r", c=3, r=8)
                            e0rv = pk[:, 24 * T2 + 8 * c0:
                                      24 * T2 + 8 * (c0 + CH)].rearrange(
                                "p (t r) -> p t r", r=8)
                            l8v = pk[:, 32 * T2 + 8 * c0:
                                     32 * T2 + 8 * (c0 + CH)].rearrange(
                                "p (t r) -> p t r", r=8)
                            if not cfg.ab_no_s4:
                                nc.vector.tensor_tensor(
                                    out=S4[:, c0 * P:(c0 + CH) * P].rearrange(
                                        "p (t v r) -> p t v r", v=16, r=8),
                                    in0=iota_bf[:].rearrange(
                                        "p (v r) -> p v r", r=8)
                                        .unsqueeze(1).broadcast_to(
                                        [P, TT, 16, 8]),
                                    in1=l8v.unsqueeze(2).broadcast_to(
                                        [P, TT, 16, 8]),
                                    op=EQ)
                            ysv = ys_all[:, c0 * DE:(c0 + CH) * DE].rearrange(
                                "p (t f) -> p t f", f=DE)
                            ys0v = ysv[:, :, 0:MUL0]
                            ys1v = ysv[:, :, MUL0:D_IN].rearrange(
                                "p t (c u) -> p t c u", u=MUL1)
                            w4v = w4_sb[:, c0 * WN:(c0 + CH) * WN].rearrange(
                                "p (t k) -> p t k", k=WN)
                            msgv = msg4[:, c0 * D_MID:(c0 + CH) * D_MID]\
                                .rearrange("p (t k) -> p t k", k=D_MID)
                            msg1 = msgv[:, :, 96:D_MID].rearrange(
                                "p t (c x) -> p t c x", x=96)
                            twv = tw[:, c0 * 2 * MUL0:(c0 + CH) * 2 * MUL0]\
                                .rearrange("p (t r u) -> p t r u", r=2, u=MUL0)
                            vtt(out=twv,
                                in0=w4v[:, :, 0:2 * MUL0].rearrange(
                                    "p t (r u) -> p t r u", u=MUL0),
                                in1=ys0v.unsqueeze(2).broadcast_to(
                                    [P, TT, 2, MUL0]),
                                op=MU)
                            vtt(out=msgv[:, :, 0:MUL0].rearrange(
                                    "p t (v r) -> p t v r", r=8),
                                in0=twv[:, :, 0, :].rearrange(
                                    "p t (v r) -> p t v r", r=8),
                                in1=e0rv.unsqueeze(2).broadcast_to(
                                    [P, TT, MUL0 // 8, 8]),
                                op=MU)
                            dmv = dm[:, c0 * 96:(c0 + CH) * 96].rearrange(
                                "p (t c u) -> p t c u", c=3, u=MUL1)
                            vtt(out=dmv.rearrange(
                                    "p t c (v r) -> p t c v r", r=8),
                                in0=ys1v.rearrange(
                                    "p t c (v r) -> p t c v r", r=8),
                                in1=a84.unsqueeze(3).broadcast_to(
                                    [P, TT, 3, MUL1 // 8, 8]),
                                op=MU)
                            dsv = ds[:, c0 * MUL1:(c0 + CH) * MUL1].rearrange(
                                "p (t u) -> p t u", u=MUL1)
                            vtt(out=dsv, in0=dmv[:, :, 0, :],
                                in1=dmv[:, :, 1, :], op=AD)
                            vtt(out=dsv, in0=dsv, in1=dmv[:, :, 2, :], op=AD)
                            vtt(out=msgv[:, :, MUL0:96], in0=dsv,
                                in1=w4v[:, :, 160:WN], op=MU)
                            t3v = t3t[:, c0 * MUL1:(c0 + CH) * MUL1].rearrange(
                                "p (t u) -> p t u", u=MUL1)
                            vtt(out=t3v.rearrange("p t (v r) -> p t v r", r=8),
                                in0=w4v[:, :, 128:160].rearrange(
                                    "p t (v r) -> p t v r", r=8),
                                in1=e0rv.unsqueeze(2).broadcast_to(
                                    [P, TT, MUL1 // 8, 8]),
                                op=MU)
                            vtt(out=msg1[:, :, :, 0:MUL0].rearrange(
                                    "p t c (v r) -> p t c v r", r=8),
                                in0=twv[:, :, 1:2, :].broadcast_to(
                                    [P, TT, 3, MUL0]).rearrange(
                                    "p t c (v r) -> p t c v r", r=8),
                                in1=a84.unsqueeze(3).broadcast_to(
                                    [P, TT, 3, MUL0 // 8, 8]),
                                op=MU)
                            vtt(out=msg1[:, :, :, MUL0:96],
                                in0=t3v.unsqueeze(2).broadcast_to(
                                    [P, TT, 3, MUL1]),
                                in1=ys1v, op=MU)

                        # scatter: one selection matmul per tile
                        # (lhsT = one-hot S4 tile; z[wl] accumulates in its
                        # own PSUM bank, one group per bank)
                        zz = [psZ.tile([P, D_MID], f32, tag=f"zt{wl}",
                                       name=f"z_{wl}", bufs=1)
                              for wl in range(2)]
                        for wl in range(2):
                            tiles = ([wl * th + t for t in range(th)]
                                     + [(2 + wl) * th + t for t in range(th)])
                            if cfg.ab_no_sel:
                                tiles = tiles[:1]
                            for i, t in enumerate(tiles):
                                nc.tensor.matmul(
                                    out=zz[wl][:],
                                    lhsT=S4[:, t * P:(t + 1) * P],
                                    rhs=msg4[:, t * D_MID:(t + 1) * D_MID],
                                    start=(i == 0),
                                    stop=(i == len(tiles) - 1))

                        # ---- node phase (per window in the pair)
                        out_sb = mp.tile([P, 2 * D_IN], f32, tag="outsb")
                        for wl in ([] if cfg.ab_no_node else range(ps)):
                            w = w0 + wl
                            z_sb = mp.tile([P, D_MID], bf, tag="zsb")
                            nc.scalar.activation(out=z_sb[:], in_=zz[wl][:],
                                                 func=CP)
                            o_ps = psN.tile([P, D_IN], f32, tag="o", bufs=1)
                            for b in range(4):
                                zT_ps = psZ.tile([96, P], bf, tag=f"zt{wl}",
                                                 name=f"zT_{wl}", bufs=1)
                                nc.tensor.transpose(
                                    out=zT_ps[:],
                                    in_=z_sb[:, b * 96:(b + 1) * 96],
                                    identity=ident[:])
                                zT_sb = mp.tile([96, P], bf, tag="zts")
                                if b % 2 == 0:
                                    nc.scalar.activation(out=zT_sb[:],
                                                         in_=zT_ps[:], func=CP)
                                else:
                                    nc.vector.tensor_copy(out=zT_sb[:],
                                                          in_=zT_ps[:])
                                if b == 0:
                                    nc.tensor.matmul(out=o_ps[:, 0:MUL0],
                                                     lhsT=zT_sb[:],
                                                     rhs=Wl20_sb[:],
                                                     start=True, stop=True)
                                else:
                                    c = b - 1
                                    nc.tensor.matmul(
                                        out=o_ps[:, MUL0 + c * MUL1:
                                                 MUL0 + (c + 1) * MUL1],
                                        lhsT=zT_sb[:], rhs=Wl21_sb[:],
                                        start=True, stop=True)
                            nc.vector.tensor_tensor(
                                out=out_sb[:, wl * D_IN:(wl + 1) * D_IN],
                                in0=o_ps[:],
                                in1=s_store[:, w * D_IN:(w + 1) * D_IN], op=AD)
                        if cfg.ab_no_node:
                            nc.scalar.activation(out=out_sb[:, 0:16],
                                                 in_=iota_bf[:, 0:16], func=CP)
                        nc.sync.dma_start(
                            out=out_d[w0 * P:(w0 + ps) * P, :]
                                .rearrange("(g p) f -> p g f", p=P),
                            in_=out_sb[:, 0:ps * D_IN]
                                .rearrange("p (g f) -> p g f", f=D_IN))

    nc.compile()
    _PROG_CACHE[cfg] = nc
    return nc


# ---------------------------------------------------------------- entry point

def _assemble(cfg: Cfg, results, node_attr):
    outs = [results[k]["out"][:cfg.npc] for k in range(cfg.n_cores)]
    o = np.concatenate(outs, axis=0).astype(F32)
    o = np.concatenate([o[:, :MUL0], _from_cmajor(o[:, MUL0:])], axis=1)
    return o * node_attr


def kernel(**inputs):
    cfg, in_maps, node_attr = _prep(inputs, n_cores=8)
    nc = _build(cfg)
    res = run_bass_kernel_spmd(nc, in_maps, core_ids=list(range(cfg.n_cores)))
    return _assemble(cfg, res.results, node_attr)
